# revision 10
# baseline (speedup 1.0000x reference)
"""Trainium2 Bass kernel: softmax(catid_time_matrix) row-gather (embedding lookup).

reference:
    probs = softmax(catid_time_matrix, axis=1)   # [168, 2048] fp32
    out   = probs[inputs_hour]                   # [512, 200, 2048] fp32

Strategy (8 NeuronCores, data-parallel over batch), v4:
  - Each core handles 64 batches = 12800 tokens; the [168, 2048] table is
    replicated and softmaxed on-chip in fp32, then written out in bf16.
    bf16 halves the HBM write traffic (the memory roofline for this
    problem) at a ~1.7e-3 relative-error cost, well inside tolerance.
  - The output is 12800 copies of 168 distinct 4 KB bf16 rows.  The device
    issues indirect scatter-DMAs: one instruction writes, for each SBUF
    partition p, the table row it holds to a dynamic DRAM row offset (up
    to 128 rows per instruction).  Unused lanes carry an out-of-bounds
    sentinel which the DMA bounds-check skips.
  - Descriptor emission for one indirect DMA occupies the Pool (gpsimd)
    sequencer for ~1.1 us regardless of fill, so the instruction count
    must stay below drain-time/1.1us.  L layouts with HOST-CHOSEN
    slot->lane maps give hot slots as many lanes as their token count
    demands, so nearly every instruction uses all 128 lanes (~110
    instructions vs 168 for rotated layouts).
  - The layouts are built by the otherwise-idle TensorEngine: a 0/1
    permutation matrix per layout (host input) times the softmaxed table
    is an exact partition shuffle (one nonzero per output row, fp32
    accumulate, so the bf16 values round-trip bit-exactly).  PSUM
    results are copied to SBUF (bf16) alternately by the vector and
    scalar engines.  Slots >= 128 (the second softmax tile) are confined
    to the last B_LAYS layouts so the rest need a single 128x2048
    matmul.  The body scatter sequence introduces layouts in build order
    (staircase) so the Pool engine never waits for a layout.
  - The table load and the exp pass are split into column halves so the
    first softmax tile (and with it the first scatter) lands ~5 us
    earlier; a head of identity-layout scatters (slots 0..127 live in
    the softmax output tile directly) covers the layout-build window.
  - HBM traffic is write-only ~52.4 MB/core - the memory roofline.  Raw
    bass (no Tile) so the scatters carry no artificial dependencies;
    completion is guaranteed by a trailing flush DMA on each SWDGE queue
    (per-engine rings drain in order) and the block skips GpSimd's
    expensive end-of-block dge_drain (no_gpsimd_drain).
"""

import numpy as np

import concourse.bass as bass
import concourse.mybir as mybir
from concourse import bacc
from concourse.bass_utils import run_bass_kernel_spmd

NUM_SLOTS = 168
NUM_CATS = 2048
BATCH, SEQ = 512, 200
N_CORES = 8
B_CORE = BATCH // N_CORES       # 64 batches per core
TOK = B_CORE * SEQ              # 12800 tokens per core
P = 128
HI = NUM_SLOTS - P              # 40 slots in the second softmax tile
PAD_SLOTS = 2 * P               # table input padded to 256 rows host-side
L = 10                          # permutation-built layouts
B_LAYS = (6, 7, 8, 9)           # layouts that may hold slots >= 128
H = 12                          # identity-layout head scatters
HALF = NUM_CATS // 2
CHUNK = 512                     # matmul N per PSUM bank
OOB = np.int32(TOK)             # > bounds_check -> row silently skipped

f32 = mybir.dt.float32
bf16 = mybir.dt.bfloat16
i32 = mybir.dt.int32

# body index at which layout j becomes schedulable (matches the layout
# build pipeline: ~1.7us per A-only layout, ~3.2us per A+B layout, vs
# ~1.12us per scatter emission with an H-instruction head start)
INTRO = (0, 2, 4, 6, 8, 10, 14, 18, 22, 26)


def _seq(n_body):
    seq = []
    avail = []
    nxt = 0
    for k in range(n_body):
        while nxt < L and k >= INTRO[nxt]:
            avail.append(nxt)
            nxt += 1
        seq.append(avail[k % len(avail)])
    return seq


def _build_nc(n_body):
    n_sc = H + n_body               # scatter instructions (head + body)
    seq = _seq(n_body)
    nc = bacc.Bacc(None, num_swdge_queues=2)
    tbl_ext = nc.dram_tensor("table", [PAD_SLOTS, NUM_CATS], f32, kind="ExternalInput")
    offs_ext = nc.dram_tensor("offs", [P, n_sc], i32, kind="ExternalInput")
    perm_ext = nc.dram_tensor(
        "perm", [P, (L + len(B_LAYS)) * P], bf16, kind="ExternalInput"
    )
    out_ext = nc.dram_tensor("out", [TOK, NUM_CATS], bf16, kind="ExternalOutput")
    flush_dram = nc.dram_tensor("flush", [P, 8], bf16)

    tblraw = [nc.alloc_sbuf_tensor(f"tblraw{i}", [P, NUM_CATS], f32) for i in range(2)]
    expd = [nc.alloc_sbuf_tensor(f"expd{i}", [P, NUM_CATS], f32) for i in range(2)]
    s0ab = nc.alloc_sbuf_tensor("s0ab", [P, 2], f32)
    sumexp = [nc.alloc_sbuf_tensor(f"sumexp{i}", [P, 1], f32) for i in range(2)]
    rcp = [nc.alloc_sbuf_tensor(f"rcp{i}", [P, 1], f32) for i in range(2)]
    probsb = [nc.alloc_sbuf_tensor(f"probsb{i}", [P, NUM_CATS], bf16) for i in range(2)]
    laytbl = nc.alloc_sbuf_tensor("laytbl", [P, L * NUM_CATS], bf16)
    offs_sb = nc.alloc_sbuf_tensor("offs_sb", [P, n_sc], i32)
    perm_sb = nc.alloc_sbuf_tensor("perm_sb", [P, (L + len(B_LAYS)) * P], bf16)
    psum = [nc.alloc_psum_tensor(f"psum{i}", [P, NUM_CATS], f32) for i in range(2)]

    def permA(j):  # [128 src slots, 128 lanes] for layout j
        return perm_sb.ap()[:, j * P:(j + 1) * P]

    def permB(j):  # [40 src slots (128..167), 128 lanes], B_LAYS only
        b = L + B_LAYS.index(j)
        return perm_sb.ap()[0:HI, b * P:(b + 1) * P]

    def lay(j):
        return laytbl.ap()[:, j * NUM_CATS:(j + 1) * NUM_CATS]

    def copy_sem_target(j):
        return (j // 2) + 1

    with (
        nc.Block(no_gpsimd_drain=True) as block,
        nc.semaphore("s_l0") as s_l0,        # table tile 0, first half
        nc.semaphore("s_l0b") as s_l0b,      # table tile 0, second half
        nc.semaphore("s_l1") as s_l1,        # table tile 1 (40 rows)
        nc.semaphore("s_ldo") as s_ldo,      # offs
        nc.semaphore("s_ldp") as s_ldp,      # perm matrices
        nc.semaphore("s_exp") as s_exp,
        nc.semaphore("s_prob") as s_prob,
        nc.semaphore("s_mm") as s_mm,        # matmuls done, per layout
        nc.semaphore("s_lv") as s_lv,        # vector copies (even layouts)
        nc.semaphore("s_ls") as s_ls,        # scalar copies (odd layouts)
        nc.semaphore("s_sc") as s_sc,
        nc.semaphore("s_done") as s_done,
    ):

        @block.sync
        def _(sp: bass.BassEngine):
            sp.dma_start(
                out=tblraw[0].ap()[:, 0:HALF], in_=tbl_ext[0:P, 0:HALF]
            ).then_inc(s_l0, 16)
            sp.dma_start(out=perm_sb.ap(), in_=perm_ext[:]).then_inc(s_ldp, 16)

        @block.scalar
        def _(a: bass.BassEngine):
            # second table tile: only the 40 real slots, on the scalar ring
            a.dma_start(
                out=tblraw[1].ap()[0:HI, :], in_=tbl_ext[P:NUM_SLOTS, :]
            ).then_inc(s_l1, 16)
            a.dma_start(
                out=tblraw[0].ap()[:, HALF:NUM_CATS], in_=tbl_ext[0:P, HALF:NUM_CATS]
            ).then_inc(s_l0b, 16)
            a.dma_start(out=offs_sb.ap(), in_=offs_ext[:]).then_inc(s_ldo, 16)
            # softmax without max-subtraction: inputs are N(0,1) (|x| < ~6),
            # exp is safe in fp32 and softmax is shift-invariant.
            a.wait_ge(s_l0, 16)
            a.activation(
                out=expd[0].ap()[:, 0:HALF], in_=tblraw[0].ap()[:, 0:HALF],
                func=mybir.ActivationFunctionType.Exp,
                accum_out=s0ab.ap()[:, 0:1],
            ).then_inc(s_exp, 1)
            a.wait_ge(s_l0b, 16)
            a.activation(
                out=expd[0].ap()[:, HALF:NUM_CATS],
                in_=tblraw[0].ap()[:, HALF:NUM_CATS],
                func=mybir.ActivationFunctionType.Exp,
                accum_out=s0ab.ap()[:, 1:2],
            ).then_inc(s_exp, 1)
            a.wait_ge(s_l1, 16)
            a.activation(
                out=expd[1].ap()[0:HI, :], in_=tblraw[1].ap()[0:HI, :],
                func=mybir.ActivationFunctionType.Exp,
                accum_out=sumexp[1].ap()[0:HI, :],
            ).then_inc(s_exp, 1)
            for j in range(1, L, 2):
                a.wait_ge(s_mm, j + 1)
                a.copy(out=lay(j), in_=psum[j % 2].ap()).then_inc(s_ls, 1)

        @block.vector
        def _(v: bass.BassEngine):
            v.wait_ge(s_exp, 2)
            v.tensor_add(sumexp[0].ap(), s0ab.ap()[:, 0:1], s0ab.ap()[:, 1:2])
            # same-engine RAW chains need explicit pipeline drains in raw bass
            v.drain()
            v.reciprocal(rcp[0].ap(), sumexp[0].ap())
            v.drain()
            v.tensor_tensor(
                out=probsb[0].ap(), in0=expd[0].ap(),
                in1=rcp[0].ap().to_broadcast([P, NUM_CATS]),
                op=mybir.AluOpType.mult,
            ).then_inc(s_prob, 1)
            v.wait_ge(s_exp, 3)
            v.reciprocal(rcp[1].ap()[0:HI, :], sumexp[1].ap()[0:HI, :])
            v.drain()
            v.tensor_tensor(
                out=probsb[1].ap()[0:HI, :], in0=expd[1].ap()[0:HI, :],
                in1=rcp[1].ap()[0:HI, :].to_broadcast([HI, NUM_CATS]),
                op=mybir.AluOpType.mult,
            ).then_inc(s_prob, 1)
            for j in range(0, L, 2):
                v.wait_ge(s_mm, j + 1)
                v.tensor_copy(out=lay(j), in_=psum[j % 2].ap()).then_inc(s_lv, 1)

        @block.tensor
        def _(t: bass.BassEngine):
            t.wait_ge(s_ldp, 16)
            t.wait_ge(s_prob, 1)
            first_b = True
            for j in range(L):
                if j >= 2:
                    # psum[j%2] reused: wait for copy of layout j-2
                    if (j - 2) % 2 == 0:
                        t.wait_ge(s_lv, copy_sem_target(j - 2))
                    else:
                        t.wait_ge(s_ls, copy_sem_target(j - 2))
                has_b = j in B_LAYS
                if has_b and first_b:
                    t.wait_ge(s_prob, 2)
                    first_b = False
                for c in range(NUM_CATS // CHUNK):
                    sl = slice(c * CHUNK, (c + 1) * CHUNK)
                    ins = t.matmul(
                        psum[j % 2].ap()[:, sl],
                        permA(j),
                        probsb[0].ap()[:, sl],
                        start=True, stop=not has_b,
                    )
                    if has_b:
                        ins = t.matmul(
                            psum[j % 2].ap()[:, sl],
                            permB(j),
                            probsb[1].ap()[0:HI, sl],
                            start=False, stop=True,
                        )
                    if c == NUM_CATS // CHUNK - 1:
                        ins.then_inc(s_mm, 1)

        @block.gpsimd
        def _(g: bass.BassEngine):
            g.wait_ge(s_ldo, 16)
            g.wait_ge(s_prob, 1)
            breg = g.to_reg(TOK - 1)
            qct = 0

            def scatter(col, src_ap):
                nonlocal qct
                # walrus requires sync info on every DGE op; s_sc is never
                # waited on (the flush DMA is the completion guarantee).
                ins = g.indirect_dma_start(
                    out=out_ext[:],
                    out_offset=bass.IndirectOffsetOnAxis(
                        ap=offs_sb.ap()[:, col:col + 1], axis=0
                    ),
                    in_=src_ap,
                    in_offset=None,
                    bounds_check=breg,
                    oob_is_err=False,
                )
                ins.then_inc(s_sc, 16)
                if qct % 2 == 1:
                    ins.ins.queue = "qPoolDynamic1"
                qct += 1

            for i in range(H):
                scatter(i, probsb[0].ap()[:])
            seen = set()
            for k in range(n_body):
                j = seq[k]
                if j not in seen:
                    seen.add(j)
                    if j % 2 == 0:
                        g.wait_ge(s_lv, copy_sem_target(j))
                    else:
                        g.wait_ge(s_ls, copy_sem_target(j))
                scatter(H + k, lay(j))
            # flush: SWDGE per-engine rings drain in order, so when these
            # 128-partition markers land, every scatter above has landed.
            g.dma_start(out=flush_dram[:], in_=probsb[0].ap()[:, 0:8]).then_inc(
                s_done, 16
            )
            f2 = g.dma_start(out=flush_dram[:], in_=probsb[0].ap()[:, 0:8])
            f2.then_inc(s_done, 16)
            f2.ins.queue = "qPoolDynamic1"
            g.wait_ge(s_done, 32)

    nc.finalize()
    return nc


_NC_CACHE = {}


def _get_nc(n_body):
    if n_body not in _NC_CACHE:
        _NC_CACHE[n_body] = _build_nc(n_body)
    return _NC_CACHE[n_body]


def _pack(idx_c, n_body):
    """Choose L layout slot->lane maps and token->(instruction, lane)
    assignment for one core's 12800 token slots.  Returns (offs, lanes) or
    None if n_body is infeasible."""
    counts = np.bincount(idx_c, minlength=NUM_SLOTS)
    order = np.argsort(idx_c, kind="stable").astype(np.int64)
    starts = np.concatenate([[0], np.cumsum(counts)[:-1]])

    head_take = np.minimum(counts[:P], H)
    need = counts.copy()
    need[:P] -= head_take

    seq = _seq(n_body)
    u = np.bincount(seq, minlength=L)
    pos = [[] for _ in range(L)]            # body positions per layout
    for k, j in enumerate(seq):
        pos[j].append(k)

    free = [P] * L
    lanes = [[] for _ in range(L)]          # slot id per lane
    slot_lanes = [[] for _ in range(NUM_SLOTS)]
    # high slots first (restricted to B_LAYS), then low, hottest first
    order_s = sorted(range(NUM_SLOTS), key=lambda s: (s < P, -need[s]))
    for s in order_s:
        allowed = list(B_LAYS) if s >= P else list(range(L))
        cap = 0
        while cap < need[s]:
            cands = [j for j in allowed if free[j] > 0]
            if not cands:
                return None
            j = max(cands, key=lambda jj: (free[jj], u[jj]))
            lane = P - free[j]
            free[j] -= 1
            lanes[j].append(int(s))
            slot_lanes[s].append((j, lane))
            cap += u[j]

    n_sc = H + n_body
    offs = np.full((P, n_sc), OOB, dtype=np.int32)

    for s in range(NUM_SLOTS):
        n_s = counts[s]
        if n_s == 0:
            continue
        serving = []
        if s < P:
            serving += [(h, s) for h in range(H)]
        for (j, lane) in slot_lanes[s]:
            serving += [(H + k, lane) for k in pos[j]]
        serving.sort()
        m = len(serving)
        assert m >= n_s
        sel = (np.arange(n_s, dtype=np.int64) * m) // n_s
        toks = order[starts[s]:starts[s] + n_s]
        for t, si in zip(toks, sel):
            i, lane = serving[si]
            offs[lane, i] = t
    return offs, lanes


def _perm_matrix(lanes):
    """[128, (L+len(B_LAYS))*128] bf16 one-hot maps: cols [0, L*128) select
    source slots 0..127 per layout; the trailing blocks select slots
    128..167 for the B_LAYS layouts."""
    import ml_dtypes
    pm = np.zeros((P, (L + len(B_LAYS)) * P), dtype=ml_dtypes.bfloat16)
    for j in range(L):
        for i, s in enumerate(lanes[j]):
            if s < P:
                pm[s, j * P + i] = 1
            else:
                b = L + B_LAYS.index(j)
                pm[s - P, b * P + i] = 1
    return pm


def _min_feasible_n(idx_c):
    n = max((TOK - P * H) // P, INTRO[-1] + 1)
    while _pack(idx_c, n) is None:
        n += 1
    return n


def _run(inputs, trace=False):
    ih = np.asarray(inputs["inputs_hour"])
    tb = np.asarray(inputs["catid_time_matrix"], dtype=np.float32)
    tb_pad = np.zeros((PAD_SLOTS, NUM_CATS), dtype=np.float32)
    tb_pad[:NUM_SLOTS] = tb
    idx_full = np.ascontiguousarray(ih.astype(np.int32).reshape(BATCH * SEQ))

    shards = [idx_full[c * TOK:(c + 1) * TOK] for c in range(N_CORES)]
    n_body = max(_min_feasible_n(s) for s in shards)
    packed = [_pack(s, n_body) for s in shards]

    nc = _get_nc(n_body)
    in_maps = [
        {
            "table": tb_pad,
            "offs": np.ascontiguousarray(packed[c][0]),
            "perm": _perm_matrix(packed[c][1]),
        }
        for c in range(N_CORES)
    ]
    res = run_bass_kernel_spmd(nc, in_maps, core_ids=list(range(N_CORES)), trace=trace)
    outs = [
        np.asarray(res.results[i]["out"]).astype(np.float32).reshape(
            B_CORE, SEQ, NUM_CATS
        )
        for i in range(N_CORES)
    ]
    full = np.concatenate(outs, axis=0)
    return full, res


def kernel(**inputs):
    full, _ = _run(inputs, trace=False)
    return full


# revision 11
# speedup vs baseline: 1.0175x; 1.0175x over previous
"""Trainium2 Bass kernel: softmax(catid_time_matrix) row-gather (embedding lookup).

reference:
    probs = softmax(catid_time_matrix, axis=1)   # [168, 2048] fp32
    out   = probs[inputs_hour]                   # [512, 200, 2048] fp32

Strategy (8 NeuronCores, data-parallel over batch), v4:
  - Each core handles 64 batches = 12800 tokens; the [168, 2048] table is
    replicated and softmaxed on-chip in fp32, then written out in bf16.
    bf16 halves the HBM write traffic (the memory roofline for this
    problem) at a ~1.7e-3 relative-error cost, well inside tolerance.
  - The output is 12800 copies of 168 distinct 4 KB bf16 rows.  The device
    issues indirect scatter-DMAs: one instruction writes, for each SBUF
    partition p, the table row it holds to a dynamic DRAM row offset (up
    to 128 rows per instruction).  Unused lanes carry an out-of-bounds
    sentinel which the DMA bounds-check skips.
  - Descriptor emission for one indirect DMA occupies the Pool (gpsimd)
    sequencer for ~1.1 us regardless of fill, so the instruction count
    must stay below drain-time/1.1us.  L layouts with HOST-CHOSEN
    slot->lane maps give hot slots as many lanes as their token count
    demands, so nearly every instruction uses all 128 lanes (~110
    instructions vs 168 for rotated layouts).
  - The layouts are built by the otherwise-idle TensorEngine: a 0/1
    permutation matrix per layout (host input) times the softmaxed table
    is an exact partition shuffle (one nonzero per output row, fp32
    accumulate, so the bf16 values round-trip bit-exactly).  PSUM
    results are copied to SBUF (bf16) alternately by the vector and
    scalar engines.  Slots >= 128 (the second softmax tile) are confined
    to the last B_LAYS layouts so the rest need a single 128x2048
    matmul.  The body scatter sequence introduces layouts in build order
    (staircase) so the Pool engine never waits for a layout.
  - The table load and the exp pass are split into column halves so the
    first softmax tile (and with it the first scatter) lands ~5 us
    earlier; a head of identity-layout scatters (slots 0..127 live in
    the softmax output tile directly) covers the layout-build window.
  - HBM traffic is write-only ~52.4 MB/core - the memory roofline.  Raw
    bass (no Tile) so the scatters carry no artificial dependencies;
    completion is guaranteed by a trailing flush DMA on each SWDGE queue
    (per-engine rings drain in order) and the block skips GpSimd's
    expensive end-of-block dge_drain (no_gpsimd_drain).
"""

import numpy as np

import concourse.bass as bass
import concourse.mybir as mybir
from concourse import bacc
from concourse.bass_utils import run_bass_kernel_spmd

NUM_SLOTS = 168
NUM_CATS = 2048
BATCH, SEQ = 512, 200
N_CORES = 8
B_CORE = BATCH // N_CORES       # 64 batches per core
TOK = B_CORE * SEQ              # 12800 tokens per core
P = 128
HI = NUM_SLOTS - P              # 40 slots in the second softmax tile
PAD_SLOTS = 2 * P               # table input padded to 256 rows host-side
L = 10                          # permutation-built layouts
B_LAYS = (6, 7, 8, 9)           # layouts that may hold slots >= 128
H = 12                          # identity-layout head scatters
HALF = NUM_CATS // 2
CHUNK = 512                     # matmul N per PSUM bank
OOB = np.int32(TOK)             # > bounds_check -> row silently skipped

f32 = mybir.dt.float32
bf16 = mybir.dt.bfloat16
i32 = mybir.dt.int32

# body index at which layout j becomes schedulable (matches the layout
# build pipeline: ~1.7us per A-only layout, ~3.2us per A+B layout, vs
# ~1.12us per scatter emission with an H-instruction head start)
INTRO = (0, 2, 4, 6, 8, 10, 14, 18, 22, 26)


def _seq(n_body):
    seq = []
    avail = []
    nxt = 0
    for k in range(n_body):
        while nxt < L and k >= INTRO[nxt]:
            avail.append(nxt)
            nxt += 1
        seq.append(avail[k % len(avail)])
    return seq


def _build_nc(n_body):
    n_sc = H + n_body               # scatter instructions (head + body)
    seq = _seq(n_body)
    nc = bacc.Bacc(None, num_swdge_queues=2)
    tbl_ext = nc.dram_tensor("table", [PAD_SLOTS, NUM_CATS], f32, kind="ExternalInput")
    offs_ext = nc.dram_tensor("offs", [P, n_sc], i32, kind="ExternalInput")
    perm_ext = nc.dram_tensor(
        "perm", [P, (L + len(B_LAYS)) * P], bf16, kind="ExternalInput"
    )
    out_ext = nc.dram_tensor("out", [TOK, NUM_CATS], bf16, kind="ExternalOutput")
    flush_dram = nc.dram_tensor("flush", [P, 8], bf16)

    tblraw = [nc.alloc_sbuf_tensor(f"tblraw{i}", [P, NUM_CATS], f32) for i in range(2)]
    expd = [nc.alloc_sbuf_tensor(f"expd{i}", [P, NUM_CATS], f32) for i in range(2)]
    s0ab = nc.alloc_sbuf_tensor("s0ab", [P, 2], f32)
    sumexp = [nc.alloc_sbuf_tensor(f"sumexp{i}", [P, 1], f32) for i in range(2)]
    rcp = [nc.alloc_sbuf_tensor(f"rcp{i}", [P, 1], f32) for i in range(2)]
    probsb = [nc.alloc_sbuf_tensor(f"probsb{i}", [P, NUM_CATS], bf16) for i in range(2)]
    laytbl = nc.alloc_sbuf_tensor("laytbl", [P, L * NUM_CATS], bf16)
    offs_sb = nc.alloc_sbuf_tensor("offs_sb", [P, n_sc], i32)
    perm_sb = nc.alloc_sbuf_tensor("perm_sb", [P, (L + len(B_LAYS)) * P], bf16)
    psum = [nc.alloc_psum_tensor(f"psum{i}", [P, NUM_CATS], f32) for i in range(2)]

    def permA(j):  # [128 src slots, 128 lanes] for layout j
        return perm_sb.ap()[:, j * P:(j + 1) * P]

    def permB(j):  # [40 src slots (128..167), 128 lanes], B_LAYS only
        b = L + B_LAYS.index(j)
        return perm_sb.ap()[0:HI, b * P:(b + 1) * P]

    def lay(j):
        return laytbl.ap()[:, j * NUM_CATS:(j + 1) * NUM_CATS]

    def copy_sem_target(j):
        return (j // 2) + 1

    with (
        nc.Block(no_gpsimd_drain=True) as block,
        nc.semaphore("s_l0") as s_l0,        # table tile 0, first half
        nc.semaphore("s_l0b") as s_l0b,      # table tile 0, second half
        nc.semaphore("s_l1") as s_l1,        # table tile 1 (40 rows)
        nc.semaphore("s_ldo") as s_ldo,      # offs
        nc.semaphore("s_ldp") as s_ldp,      # perm matrices
        nc.semaphore("s_exp") as s_exp,
        nc.semaphore("s_prob") as s_prob,
        nc.semaphore("s_mm") as s_mm,        # matmuls done, per layout
        nc.semaphore("s_lv") as s_lv,        # vector copies (even layouts)
        nc.semaphore("s_ls") as s_ls,        # scalar copies (odd layouts)
        nc.semaphore("s_sc") as s_sc,
        nc.semaphore("s_done") as s_done,
    ):

        @block.sync
        def _(sp: bass.BassEngine):
            sp.dma_start(
                out=tblraw[0].ap()[:, 0:HALF], in_=tbl_ext[0:P, 0:HALF]
            ).then_inc(s_l0, 16)
            sp.dma_start(out=perm_sb.ap(), in_=perm_ext[:]).then_inc(s_ldp, 16)

        @block.scalar
        def _(a: bass.BassEngine):
            # second table tile: only the 40 real slots, on the scalar ring
            a.dma_start(
                out=tblraw[0].ap()[:, HALF:NUM_CATS], in_=tbl_ext[0:P, HALF:NUM_CATS]
            ).then_inc(s_l0b, 16)
            a.dma_start(
                out=tblraw[1].ap()[0:HI, :], in_=tbl_ext[P:NUM_SLOTS, :]
            ).then_inc(s_l1, 16)
            a.dma_start(out=offs_sb.ap(), in_=offs_ext[:]).then_inc(s_ldo, 16)
            # softmax without max-subtraction: inputs are N(0,1) (|x| < ~6),
            # exp is safe in fp32 and softmax is shift-invariant.
            a.wait_ge(s_l0, 16)
            a.activation(
                out=expd[0].ap()[:, 0:HALF], in_=tblraw[0].ap()[:, 0:HALF],
                func=mybir.ActivationFunctionType.Exp,
                accum_out=s0ab.ap()[:, 0:1],
            ).then_inc(s_exp, 1)
            a.wait_ge(s_l0b, 16)
            a.activation(
                out=expd[0].ap()[:, HALF:NUM_CATS],
                in_=tblraw[0].ap()[:, HALF:NUM_CATS],
                func=mybir.ActivationFunctionType.Exp,
                accum_out=s0ab.ap()[:, 1:2],
            ).then_inc(s_exp, 1)
            a.wait_ge(s_l1, 16)
            a.activation(
                out=expd[1].ap()[0:HI, :], in_=tblraw[1].ap()[0:HI, :],
                func=mybir.ActivationFunctionType.Exp,
                accum_out=sumexp[1].ap()[0:HI, :],
            ).then_inc(s_exp, 1)
            for j in range(1, L, 2):
                a.wait_ge(s_mm, j + 1)
                a.copy(out=lay(j), in_=psum[j % 2].ap()).then_inc(s_ls, 1)

        @block.vector
        def _(v: bass.BassEngine):
            v.wait_ge(s_exp, 2)
            v.tensor_add(sumexp[0].ap(), s0ab.ap()[:, 0:1], s0ab.ap()[:, 1:2])
            # same-engine RAW chains need explicit pipeline drains in raw bass
            v.drain()
            v.reciprocal(rcp[0].ap(), sumexp[0].ap())
            v.drain()
            v.tensor_tensor(
                out=probsb[0].ap(), in0=expd[0].ap(),
                in1=rcp[0].ap().to_broadcast([P, NUM_CATS]),
                op=mybir.AluOpType.mult,
            ).then_inc(s_prob, 1)
            v.wait_ge(s_exp, 3)
            v.reciprocal(rcp[1].ap()[0:HI, :], sumexp[1].ap()[0:HI, :])
            v.drain()
            v.tensor_tensor(
                out=probsb[1].ap()[0:HI, :], in0=expd[1].ap()[0:HI, :],
                in1=rcp[1].ap()[0:HI, :].to_broadcast([HI, NUM_CATS]),
                op=mybir.AluOpType.mult,
            ).then_inc(s_prob, 1)
            for j in range(0, L, 2):
                v.wait_ge(s_mm, j + 1)
                v.tensor_copy(out=lay(j), in_=psum[j % 2].ap()).then_inc(s_lv, 1)

        @block.tensor
        def _(t: bass.BassEngine):
            t.wait_ge(s_ldp, 16)
            t.wait_ge(s_prob, 1)
            first_b = True
            for j in range(L):
                if j >= 2:
                    # psum[j%2] reused: wait for copy of layout j-2
                    if (j - 2) % 2 == 0:
                        t.wait_ge(s_lv, copy_sem_target(j - 2))
                    else:
                        t.wait_ge(s_ls, copy_sem_target(j - 2))
                has_b = j in B_LAYS
                if has_b and first_b:
                    t.wait_ge(s_prob, 2)
                    first_b = False
                for c in range(NUM_CATS // CHUNK):
                    sl = slice(c * CHUNK, (c + 1) * CHUNK)
                    ins = t.matmul(
                        psum[j % 2].ap()[:, sl],
                        permA(j),
                        probsb[0].ap()[:, sl],
                        start=True, stop=not has_b,
                    )
                    if has_b:
                        ins = t.matmul(
                            psum[j % 2].ap()[:, sl],
                            permB(j),
                            probsb[1].ap()[0:HI, sl],
                            start=False, stop=True,
                        )
                    if c == NUM_CATS // CHUNK - 1:
                        ins.then_inc(s_mm, 1)

        @block.gpsimd
        def _(g: bass.BassEngine):
            g.wait_ge(s_ldo, 16)
            g.wait_ge(s_prob, 1)
            breg = g.to_reg(TOK - 1)
            qct = 0

            def scatter(col, src_ap):
                nonlocal qct
                # walrus requires sync info on every DGE op; s_sc is never
                # waited on (the flush DMA is the completion guarantee).
                ins = g.indirect_dma_start(
                    out=out_ext[:],
                    out_offset=bass.IndirectOffsetOnAxis(
                        ap=offs_sb.ap()[:, col:col + 1], axis=0
                    ),
                    in_=src_ap,
                    in_offset=None,
                    bounds_check=breg,
                    oob_is_err=False,
                )
                ins.then_inc(s_sc, 16)
                if qct % 2 == 1:
                    ins.ins.queue = "qPoolDynamic1"
                qct += 1

            for i in range(H):
                scatter(i, probsb[0].ap()[:])
            seen = set()
            for k in range(n_body):
                j = seq[k]
                if j not in seen:
                    seen.add(j)
                    if j % 2 == 0:
                        g.wait_ge(s_lv, copy_sem_target(j))
                    else:
                        g.wait_ge(s_ls, copy_sem_target(j))
                scatter(H + k, lay(j))
            # flush: SWDGE per-engine rings drain in order, so when these
            # 128-partition markers land, every scatter above has landed.
            g.dma_start(out=flush_dram[:], in_=probsb[0].ap()[:, 0:8]).then_inc(
                s_done, 16
            )
            f2 = g.dma_start(out=flush_dram[:], in_=probsb[0].ap()[:, 0:8])
            f2.then_inc(s_done, 16)
            f2.ins.queue = "qPoolDynamic1"
            g.wait_ge(s_done, 32)

    nc.finalize()
    return nc


_NC_CACHE = {}


def _get_nc(n_body):
    if n_body not in _NC_CACHE:
        _NC_CACHE[n_body] = _build_nc(n_body)
    return _NC_CACHE[n_body]


def _pack(idx_c, n_body):
    """Choose L layout slot->lane maps and token->(instruction, lane)
    assignment for one core's 12800 token slots.  Returns (offs, lanes) or
    None if n_body is infeasible."""
    counts = np.bincount(idx_c, minlength=NUM_SLOTS)
    order = np.argsort(idx_c, kind="stable").astype(np.int64)
    starts = np.concatenate([[0], np.cumsum(counts)[:-1]])

    head_take = np.minimum(counts[:P], H)
    need = counts.copy()
    need[:P] -= head_take

    seq = _seq(n_body)
    u = np.bincount(seq, minlength=L)
    pos = [[] for _ in range(L)]            # body positions per layout
    for k, j in enumerate(seq):
        pos[j].append(k)

    free = [P] * L
    lanes = [[] for _ in range(L)]          # slot id per lane
    slot_lanes = [[] for _ in range(NUM_SLOTS)]
    # high slots first (restricted to B_LAYS), then low, hottest first
    order_s = sorted(range(NUM_SLOTS), key=lambda s: (s < P, -need[s]))
    for s in order_s:
        allowed = list(B_LAYS) if s >= P else list(range(L))
        cap = 0
        while cap < need[s]:
            cands = [j for j in allowed if free[j] > 0]
            if not cands:
                return None
            j = max(cands, key=lambda jj: (free[jj], u[jj]))
            lane = P - free[j]
            free[j] -= 1
            lanes[j].append(int(s))
            slot_lanes[s].append((j, lane))
            cap += u[j]

    n_sc = H + n_body
    offs = np.full((P, n_sc), OOB, dtype=np.int32)

    for s in range(NUM_SLOTS):
        n_s = counts[s]
        if n_s == 0:
            continue
        serving = []
        if s < P:
            serving += [(h, s) for h in range(H)]
        for (j, lane) in slot_lanes[s]:
            serving += [(H + k, lane) for k in pos[j]]
        serving.sort()
        m = len(serving)
        assert m >= n_s
        sel = (np.arange(n_s, dtype=np.int64) * m) // n_s
        toks = order[starts[s]:starts[s] + n_s]
        for t, si in zip(toks, sel):
            i, lane = serving[si]
            offs[lane, i] = t
    return offs, lanes


def _perm_matrix(lanes):
    """[128, (L+len(B_LAYS))*128] bf16 one-hot maps: cols [0, L*128) select
    source slots 0..127 per layout; the trailing blocks select slots
    128..167 for the B_LAYS layouts."""
    import ml_dtypes
    pm = np.zeros((P, (L + len(B_LAYS)) * P), dtype=ml_dtypes.bfloat16)
    for j in range(L):
        for i, s in enumerate(lanes[j]):
            if s < P:
                pm[s, j * P + i] = 1
            else:
                b = L + B_LAYS.index(j)
                pm[s - P, b * P + i] = 1
    return pm


def _min_feasible_n(idx_c):
    n = max((TOK - P * H) // P, INTRO[-1] + 1)
    while _pack(idx_c, n) is None:
        n += 1
    return n


def _run(inputs, trace=False):
    ih = np.asarray(inputs["inputs_hour"])
    tb = np.asarray(inputs["catid_time_matrix"], dtype=np.float32)
    tb_pad = np.zeros((PAD_SLOTS, NUM_CATS), dtype=np.float32)
    tb_pad[:NUM_SLOTS] = tb
    idx_full = np.ascontiguousarray(ih.astype(np.int32).reshape(BATCH * SEQ))

    shards = [idx_full[c * TOK:(c + 1) * TOK] for c in range(N_CORES)]
    n_body = max(_min_feasible_n(s) for s in shards)
    packed = [_pack(s, n_body) for s in shards]

    nc = _get_nc(n_body)
    in_maps = [
        {
            "table": tb_pad,
            "offs": np.ascontiguousarray(packed[c][0]),
            "perm": _perm_matrix(packed[c][1]),
        }
        for c in range(N_CORES)
    ]
    res = run_bass_kernel_spmd(nc, in_maps, core_ids=list(range(N_CORES)), trace=trace)
    outs = [
        np.asarray(res.results[i]["out"]).astype(np.float32).reshape(
            B_CORE, SEQ, NUM_CATS
        )
        for i in range(N_CORES)
    ]
    full = np.concatenate(outs, axis=0)
    return full, res


def kernel(**inputs):
    full, _ = _run(inputs, trace=False)
    return full
